# revision 22
# baseline (speedup 1.0000x reference)
"""Euclidean distance block (retrieval kNN) on 8 TRN2 NeuronCores.

dist[b, s, p] = sqrt(sum_c (x1[b, c, p] - x2[b, s, c, p])^2)   p = spatial (h*w)
out[b] = dist[b].reshape(S * h * w)

Sharding: data-parallel over batch B=32 -> 4 batches per core, no comms.

The rel-err budget (2e-2) is far above bf16 noise (~3e-3), so x1/x2 are
cast to bf16 ON HOST and uploaded as bf16 DRAM tensors: HBM read traffic
halves (45.2 -> 22.6 MB/core) and every load becomes a plain HWDGE copy
on the sync ring (no SWDGE cast, no ~6us Q7 warmup before the first
byte). The output is stored bf16 too and upcast on host.

Per-core kernel layout: SBUF partitions carry (support_pair, channel) =
2*64 = 128; the free axis carries spatial. A big tile covers 8 supports
as [128, 4, 1764], streamed as four 451 KB pair-DMAs. Compute chain per
tile:
  DVE subtract in bf16 (2x mode), in place
  Square -> bf16: 3 slices on ACT, 1 on DVE (engine cadence balance)
  PE matmul against [128, 25] one-hot pair masks, accumulating
    per-support sums over C into a [25, 441] PSUM tile per spatial
    quarter (PSUM bank = 2 KB caps the moving width at 441)
  ACT Sqrt PSUM -> SBUF bf16, one store per batch on the Scalar HWDGE
  ring (which never carries loads, so a compute-blocked store cannot
  stall the load stream).

The Tile legalizer emits one LDWEIGHTS per matmul even when the
stationary mask repeats; _dedupe_ldweights drops the redundant reloads
(4 quarter-matmuls share one mask) so the PE streams matmuls
back-to-back.
"""

import numpy as np

B, S, C, H, W = 32, 25, 64, 42, 42
HW = H * W            # 1764
NCORES = 8
BL = B // NCORES      # 4 batches per core
NSO = 4               # support pairs per big tile (8 supports)
NBIG = 3              # big tiles per batch (24 supports), then 1 leftover
NQ = 4                # spatial quarters
# Quarter boundaries [444, 444, 444, 432]: a 441-wide quarter would start
# at 882 B (2 mod 4), which drops the PE moving-operand read to the slow
# path (~689 vs ~350 ns per matmul). 444*2 = 888 B keeps every quarter
# start 4B-aligned. Each quarter still fits a 2 KB PSUM bank.
QWID = [444, 444, 444, HW - 3 * 444]
QOFF = [0, 444, 888, 1332]
NPAIR = 13            # 12 support pairs + 1 leftover single

_cache = {}


def _dedupe_ldweights(nc):
    """Remove back-to-back-redundant InstLdweights emitted by the Tile
    legalizer (one per matmul, even when the stationary operand is
    unchanged). The 4 quarter-matmuls of a pair share one mask, so 3 of
    their 4 LDWEIGHTS reload identical weights; each costs ~115 ns on PE
    and breaks the matmul drain overlap. A matmul never name-depends on
    its own ldweights (ordering is the PE queue), ldweights carry no sem
    updates, and only the kernel's first ldweights carries a wait — so an
    ldweights with no waits whose weights AP matches the previous PE
    weight load can be dropped. Any other PE instruction conservatively
    invalidates the tracked weights."""
    import concourse.mybir as mybir

    pe = mybir.EngineType.PE
    for blk in nc.m.functions[0].blocks:
        insts = blk.instructions
        last_sig = None
        drop = set()
        for inst in insts:
            if getattr(inst, "engine", None) != pe:
                continue
            tn = type(inst).__name__
            if tn == "InstLdweights":
                sig = (
                    str(inst.ins[0]),
                    str(inst.tile_position),
                    str(inst.perf_mode),
                    str(inst.is_transpose),
                )
                si = inst.sync_info
                clean = si is None or (
                    len(si.on_wait) == 0 and len(si.on_update) == 0
                )
                if clean and sig == last_sig:
                    drop.add(id(inst))
                else:
                    last_sig = sig
            elif tn != "InstMatmult":
                last_sig = None
        if drop:
            insts[:] = [i for i in insts if id(i) not in drop]


def _build_nc():
    import concourse.bacc as bacc
    import concourse.mybir as mybir
    from concourse.tile import TileContext
    from concourse.bass import MemorySpace

    f32 = mybir.dt.float32
    bf16 = mybir.dt.bfloat16
    Square = mybir.ActivationFunctionType.Square
    Sqrt = mybir.ActivationFunctionType.Sqrt
    sub = mybir.AluOpType.subtract

    # Square and Sqrt both live in the "sqrt_and_others" act-function set,
    # but the table-load chooser picks the first set containing each one,
    # alternating two ~2.7us table reloads per batch. Strip the two
    # functions from every other set (contents only — set ids are
    # positional) so one resident table serves the whole kernel.
    _orig_tables = bacc.get_activation_tables

    def _pinned_tables(arch):
        t = _orig_tables(arch)
        for name, fns in t.items():
            if name != "sqrt_and_others":
                fns.discard(Square)
                fns.discard(Sqrt)
        return t

    bacc.get_activation_tables = _pinned_tables
    nc = bacc.Bacc()
    x1 = nc.declare_dram_parameter("x1", [BL, C, HW], bf16, isOutput=False)
    x2 = nc.declare_dram_parameter("x2", [BL, S, C, HW], bf16, isOutput=False)
    mk = nc.declare_dram_parameter("mask", [NPAIR, 128, S], bf16, isOutput=False)
    out = nc.declare_dram_parameter("out", [BL, S * HW], bf16, isOutput=True)

    with TileContext(nc) as tc:
        with (
            tc.tile_pool(name="x2p", bufs=6) as x2p,
            tc.tile_pool(name="sqp", bufs=3) as sqp,
            tc.tile_pool(name="x1p", bufs=1) as x1p,
            tc.tile_pool(name="outp", bufs=2) as outp,
            tc.tile_pool(name="cst", bufs=1) as cst,
            tc.tile_pool(name="ps", bufs=2, space=MemorySpace.PSUM) as psp,
        ):
            mt = cst.tile([128, NPAIR, S], bf16)
            nc.sync.dma_start(mt[:], mk.rearrange("g k m -> k g m"))

            # all of x1 once: [c, b, k, p] on partitions 0..63, duplicated
            # onto 64..127 AND along a pair axis k (so a two-pair subtract
            # has a plain contiguous [128, 2, HW] second operand — a
            # broadcast AP falls off DVE's packed fast path). SBUF->SBUF
            # copies ride the Scalar ring: they wait on the x1 load, and on
            # the sync ring that wait would stall the queued x2 loads ~3us.
            # b=0's slice loads first so the x2 stream starts ~1.7us sooner.
            x1all = x1p.tile([128, BL, 2, HW], bf16)
            nc.sync.dma_start(x1all[0:64, 0, 0, :], x1[0])
            nc.scalar.dma_start(x1all[64:128, 0, 0, :], x1all[0:64, 0, 0, :])
            nc.scalar.dma_start(x1all[:, 0, 1, :], x1all[:, 0, 0, :])
            nc.sync.dma_start(
                x1all[0:64, 1:BL, 0, :], x1[1:BL].rearrange("b c p -> c b p")
            )
            nc.scalar.dma_start(
                x1all[64:128, 1:BL, 0, :], x1all[0:64, 1:BL, 0, :]
            )
            nc.scalar.dma_start(x1all[:, 1:BL, 1, :], x1all[:, 1:BL, 0, :])

            # square engine per (batch, half-tile): 6 half-tiles of 2 pairs
            # per batch, 4 on ACT / 2 on DVE. Whole-tile squares were tried:
            # the coarser dependency (4 subs -> 6.2us square) starves ACT
            # mid-kernel and regressed. (GPSIMD squares also regressed:
            # ~9-17us per half AND the SBUF descriptor traffic stalls DVE's
            # 2-port packed mode.)
            SQ_ENG = ["A", "A", "D", "A", "A", "D"]

            for b in range(BL):
                # leftover support 24: DMA early so it streams with big tiles
                x2l = x2p.tile([64, HW], bf16, tag="x2l")
                nc.sync.dma_start(x2l[:], x2[b, S - 1])

                pst = [
                    psp.tile([S, QWID[q]], f32, name=f"ps{q}", tag=f"ps{q}")
                    for q in range(NQ)
                ]

                # leftover compute first keeps the end-of-batch tail short;
                # its square alternates ACT/DVE to keep the two balanced
                nc.vector.tensor_tensor(x2l[:], x2l[:], x1all[0:64, b, 0, :], sub)
                sql = sqp.tile([64, HW], bf16, name="sql", tag="sql")
                if b % 2:
                    nc.vector.tensor_tensor(
                        sql[:], x2l[:], x2l[:], mybir.AluOpType.mult
                    )
                else:
                    nc.scalar.activation(sql[:], x2l[:], Square)
                for q in range(NQ):
                    nc.tensor.matmul(
                        pst[q][:, :],
                        mt[0:64, NPAIR - 1, :],
                        sql[:, QOFF[q] : QOFF[q] + QWID[q]],
                        start=True,
                        stop=False,
                    )

                for i in range(NBIG):
                    x2t = x2p.tile([128, NSO, HW], bf16, tag="x2t")
                    x1s = x1all[:, b, 0, :]
                    src = x2[b, 8 * i : 8 * i + 8].rearrange(
                        "(so si) c p -> (si c) so p", si=2
                    )
                    # The very last tile's chain is the kernel tail: q-slice
                    # its compute so the final dependency chain is one
                    # 441-wide chunk instead of a whole 1764-wide slice.
                    last_tile = b == BL - 1 and i == NBIG - 1
                    if last_tile:
                        # finest DMA granularity so the tail chain starts
                        # on the first pair
                        for so in range(NSO):
                            nc.sync.dma_start(x2t[:, so, :], src[:, so, :])
                    else:
                        # 902KB two-pair DMAs: fewer completion sems and
                        # longer descriptor runs for the sustained stream
                        for h in range(NSO // 2):
                            nc.sync.dma_start(
                                x2t[:, 2 * h : 2 * h + 2, :],
                                src[:, 2 * h : 2 * h + 2, :],
                            )
                    ot = None
                    if last_tile:
                        ot = outp.tile([S, HW], bf16, name="ot", tag="ot")
                    sq = sqp.tile([128, NSO, HW], bf16, tag="sq")
                    if not last_tile:
                        for h in range(NSO // 2):
                            hs = slice(2 * h, 2 * h + 2)
                            # two-pair in-place subtract and square
                            nc.vector.tensor_tensor(
                                x2t[:, hs, :],
                                x2t[:, hs, :],
                                x1all[:, b, :, :],
                                sub,
                            )
                            if True:
                                if SQ_ENG[2 * i + h] == "A":
                                    nc.scalar.activation(
                                        sq[:, hs, :], x2t[:, hs, :], Square
                                    )
                                else:
                                    nc.vector.tensor_tensor(
                                        sq[:, hs, :],
                                        x2t[:, hs, :],
                                        x2t[:, hs, :],
                                        mybir.AluOpType.mult,
                                    )
                                for soj in (2 * h, 2 * h + 1):
                                    j = NSO * i + soj
                                    for q in range(NQ):
                                        nc.tensor.matmul(
                                            pst[q][:, :],
                                            mt[:, j, :],
                                            sq[:, soj, QOFF[q] : QOFF[q] + QWID[q]],
                                            start=False,
                                            stop=(j == NPAIR - 2),
                                        )
                    else:
                        for so in range(NSO):
                            j = NSO * i + so
                            for q in range(NQ):
                                qs = slice(QOFF[q], QOFF[q] + QWID[q])
                                nc.vector.tensor_tensor(
                                    x2t[:, so, qs], x2t[:, so, qs], x1s[:, qs], sub
                                )
                                if q % 2 == 0:
                                    nc.scalar.activation(
                                        sq[:, so, qs], x2t[:, so, qs], Square
                                    )
                                else:
                                    nc.vector.tensor_tensor(
                                        sq[:, so, qs],
                                        x2t[:, so, qs],
                                        x2t[:, so, qs],
                                        mybir.AluOpType.mult,
                                    )
                                nc.tensor.matmul(
                                    pst[q][:, :],
                                    mt[:, j, :],
                                    sq[:, so, qs],
                                    start=False,
                                    stop=(j == NPAIR - 2),
                                )
                                if so == NSO - 1:
                                    # quarter q is complete: sqrt + store now
                                    nc.scalar.activation(
                                        ot[:, qs], pst[q][:], Sqrt
                                    )
                                    nc.gpsimd.dma_start(
                                        out[b].rearrange("(s p) -> s p", s=S)[:, qs],
                                        ot[:, qs],
                                    )

                if b < BL - 1:
                    ot = outp.tile([S, HW], bf16, name="ot", tag="ot")
                    for q in range(NQ):
                        nc.scalar.activation(
                            ot[:, QOFF[q] : QOFF[q] + QWID[q]], pst[q][:], Sqrt
                        )
                    # store via the GpSimd SWDGE ring: it carries no loads
                    # (no stall coupling) and keeps the ~0.6us dispatch off
                    # the busy ACT queue
                    nc.gpsimd.dma_start(out[b].rearrange("(s p) -> s p", s=S), ot[:])

    try:
        _dedupe_ldweights(nc)
        nc.finalize()
    finally:
        bacc.get_activation_tables = _orig_tables
    return nc


def get_nc():
    if "nc" not in _cache:
        _cache["nc"] = _build_nc()
    return _cache["nc"]


def make_mask() -> np.ndarray:
    # mask[j, k, m] = 1 iff partition k of pair-tile j feeds output support m.
    # Pair j < 12 covers supports (2j, 2j+1): k < 64 -> 2j, k >= 64 -> 2j+1.
    # Pair 12 is the leftover single support 24 on partitions 0..63.
    import ml_dtypes

    mask = np.zeros((NPAIR, 128, S), dtype=ml_dtypes.bfloat16)
    for j in range(NPAIR - 1):
        mask[j, 0:64, 2 * j] = 1.0
        mask[j, 64:128, 2 * j + 1] = 1.0
    mask[NPAIR - 1, 0:64, S - 1] = 1.0
    return mask


def make_in_maps(x1: np.ndarray, x2: np.ndarray) -> list[dict]:
    import ml_dtypes

    bf16 = ml_dtypes.bfloat16
    x1 = np.ascontiguousarray(
        np.asarray(x1, dtype=np.float32).astype(bf16)
    ).reshape(B, C, HW)
    x2 = np.ascontiguousarray(
        np.asarray(x2, dtype=np.float32).astype(bf16)
    ).reshape(B, S, C, HW)
    mask = make_mask()
    maps = []
    for i in range(NCORES):
        sl = slice(i * BL, (i + 1) * BL)
        maps.append({"x1": x1[sl], "x2": x2[sl], "mask": mask})
    return maps


def gather_out(results: list[dict]) -> np.ndarray:
    return np.concatenate([np.asarray(r["out"]) for r in results], axis=0).astype(
        np.float32, copy=False
    )


def kernel(x1, x2) -> np.ndarray:
    from concourse.bass_utils import run_bass_kernel_spmd

    nc = get_nc()
    in_maps = make_in_maps(x1, x2)
    res = run_bass_kernel_spmd(nc, in_maps, list(range(NCORES)))
    return gather_out(res.results)


# revision 26
# speedup vs baseline: 1.1098x; 1.1098x over previous
"""Euclidean distance block (retrieval kNN) on 8 TRN2 NeuronCores.

dist[b, s, p] = sqrt(sum_c (x1[b, c, p] - x2[b, s, c, p])^2)   p = spatial (h*w)
out[b] = dist[b].reshape(S * h * w)

Sharding: data-parallel over batch B=32 -> 4 batches per core, no comms.

The rel-err budget (2e-2) is far above bf16 noise (~3e-3), so x1/x2 are
cast to bf16 ON HOST and uploaded as bf16 DRAM tensors: HBM read traffic
halves (45.2 -> 22.6 MB/core) and every load becomes a plain HWDGE copy
on the sync ring (no SWDGE cast, no ~6us Q7 warmup before the first
byte). The output is stored bf16 too and upcast on host.

Per-core kernel layout: SBUF partitions carry (support_pair, channel) =
2*64 = 128; the free axis carries spatial. A big tile covers 8 supports
as [128, 4, 1764], streamed as four 451 KB pair-DMAs. Compute chain per
tile:
  DVE subtract in bf16 (2x mode), in place
  Square -> bf16: 3 slices on ACT, 1 on DVE (engine cadence balance)
  PE matmul against [128, 25] one-hot pair masks, accumulating
    per-support sums over C into a [25, 441] PSUM tile per spatial
    quarter (PSUM bank = 2 KB caps the moving width at 441)
  ACT Sqrt PSUM -> SBUF bf16, one store per batch on the Scalar HWDGE
  ring (which never carries loads, so a compute-blocked store cannot
  stall the load stream).

The Tile legalizer emits one LDWEIGHTS per matmul even when the
stationary mask repeats; _dedupe_ldweights drops the redundant reloads
(4 quarter-matmuls share one mask) so the PE streams matmuls
back-to-back.
"""

import numpy as np

B, S, C, H, W = 32, 25, 64, 42, 42
HW = H * W            # 1764
NCORES = 8
BL = B // NCORES      # 4 batches per core
NSO = 4               # support pairs per big tile (8 supports)
NBIG = 3              # big tiles per batch (24 supports), then 1 leftover
NQ = 4                # spatial quarters
# Quarter boundaries [444, 444, 444, 432]: a 441-wide quarter would start
# at 882 B (2 mod 4), which drops the PE moving-operand read to the slow
# path (~689 vs ~350 ns per matmul). 444*2 = 888 B keeps every quarter
# start 4B-aligned. Each quarter still fits a 2 KB PSUM bank.
QWID = [444, 444, 444, HW - 3 * 444]
QOFF = [0, 444, 888, 1332]
NPAIR = 13            # 12 support pairs + 1 leftover single

_cache = {}


def _dedupe_ldweights(nc):
    """Remove back-to-back-redundant InstLdweights emitted by the Tile
    legalizer (one per matmul, even when the stationary operand is
    unchanged). The 4 quarter-matmuls of a pair share one mask, so 3 of
    their 4 LDWEIGHTS reload identical weights; each costs ~115 ns on PE
    and breaks the matmul drain overlap. A matmul never name-depends on
    its own ldweights (ordering is the PE queue), ldweights carry no sem
    updates, and only the kernel's first ldweights carries a wait — so an
    ldweights with no waits whose weights AP matches the previous PE
    weight load can be dropped. Any other PE instruction conservatively
    invalidates the tracked weights."""
    import concourse.mybir as mybir

    pe = mybir.EngineType.PE
    for blk in nc.m.functions[0].blocks:
        insts = blk.instructions
        last_sig = None
        drop = set()
        for inst in insts:
            if getattr(inst, "engine", None) != pe:
                continue
            tn = type(inst).__name__
            if tn == "InstLdweights":
                sig = (
                    str(inst.ins[0]),
                    str(inst.tile_position),
                    str(inst.perf_mode),
                    str(inst.is_transpose),
                )
                si = inst.sync_info
                clean = si is None or (
                    len(si.on_wait) == 0 and len(si.on_update) == 0
                )
                if clean and sig == last_sig:
                    drop.add(id(inst))
                else:
                    last_sig = sig
            elif tn != "InstMatmult":
                last_sig = None
        if drop:
            insts[:] = [i for i in insts if id(i) not in drop]


def _build_nc():
    import concourse.bacc as bacc
    import concourse.mybir as mybir
    from concourse.tile import TileContext
    from concourse.bass import MemorySpace

    f32 = mybir.dt.float32
    bf16 = mybir.dt.bfloat16
    Square = mybir.ActivationFunctionType.Square
    Sqrt = mybir.ActivationFunctionType.Sqrt
    sub = mybir.AluOpType.subtract

    # Square and Sqrt both live in the "sqrt_and_others" act-function set,
    # but the table-load chooser picks the first set containing each one,
    # alternating two ~2.7us table reloads per batch. Strip the two
    # functions from every other set (contents only — set ids are
    # positional) so one resident table serves the whole kernel.
    _orig_tables = bacc.get_activation_tables

    def _pinned_tables(arch):
        t = _orig_tables(arch)
        for name, fns in t.items():
            if name != "sqrt_and_others":
                fns.discard(Square)
                fns.discard(Sqrt)
        return t

    bacc.get_activation_tables = _pinned_tables
    nc = bacc.Bacc()
    x1 = nc.declare_dram_parameter("x1", [BL, C, HW], bf16, isOutput=False)
    x2 = nc.declare_dram_parameter("x2", [BL, S, C, HW], bf16, isOutput=False)
    mk = nc.declare_dram_parameter("mask", [NPAIR, 128, S], bf16, isOutput=False)
    out = nc.declare_dram_parameter("out", [BL, S * HW], bf16, isOutput=True)

    with TileContext(nc) as tc:
        with (
            tc.tile_pool(name="x2p", bufs=6) as x2p,
            tc.tile_pool(name="sqp", bufs=3) as sqp,
            tc.tile_pool(name="x1p", bufs=1) as x1p,
            tc.tile_pool(name="outp", bufs=2) as outp,
            tc.tile_pool(name="cst", bufs=1) as cst,
            tc.tile_pool(name="ps", bufs=2, space=MemorySpace.PSUM) as psp,
        ):
            mt = cst.tile([128, NPAIR, S], bf16)
            nc.sync.dma_start(mt[:], mk.rearrange("g k m -> k g m"))

            # all of x1 once: [c, b, p] on partitions 0..63, then duplicate
            # onto 64..127 via SBUF->SBUF (no extra HBM traffic). The
            # duplicate waits on the x1 load, so it rides the Scalar ring —
            # on the sync ring it would stall the queued x2 loads ~3us.
            # b=0's slice loads first so the x2 stream starts ~1.7us sooner;
            # the rest follows interleaved with b0's tiles.
            x1all = x1p.tile([128, BL, HW], bf16)
            nc.sync.dma_start(x1all[0:64, 0, :], x1[0])
            nc.scalar.dma_start(x1all[64:128, 0, :], x1all[0:64, 0, :])
            nc.sync.dma_start(
                x1all[0:64, 1:BL, :], x1[1:BL].rearrange("b c p -> c b p")
            )
            nc.scalar.dma_start(x1all[64:128, 1:BL, :], x1all[0:64, 1:BL, :])

            # square engine per (batch, half-tile): 6 half-tiles of 2 pairs
            # per batch, 4 on ACT / 2 on DVE. Whole-tile squares were tried:
            # the coarser dependency (4 subs -> 6.2us square) starves ACT
            # mid-kernel and regressed. (GPSIMD squares also regressed:
            # ~9-17us per half AND the SBUF descriptor traffic stalls DVE's
            # 2-port packed mode.)
            SQ_ENG = ["A", "A", "D", "A", "A", "D"]

            for b in range(BL):
                # leftover support 24: DMA early so it streams with big tiles
                x2l = x2p.tile([64, HW], bf16, tag="x2l")
                nc.sync.dma_start(x2l[:], x2[b, S - 1])

                pst = [
                    psp.tile([S, QWID[q]], f32, name=f"ps{q}", tag=f"ps{q}")
                    for q in range(NQ)
                ]

                # leftover compute first keeps the end-of-batch tail short
                nc.vector.tensor_tensor(x2l[:], x2l[:], x1all[0:64, b, :], sub)
                sql = sqp.tile([64, HW], bf16, name="sql", tag="sql")
                nc.scalar.activation(sql[:], x2l[:], Square)
                for q in range(NQ):
                    nc.tensor.matmul(
                        pst[q][:, :],
                        mt[0:64, NPAIR - 1, :],
                        sql[:, QOFF[q] : QOFF[q] + QWID[q]],
                        start=True,
                        stop=False,
                    )

                for i in range(NBIG):
                    x2t = x2p.tile([128, NSO, HW], bf16, tag="x2t")
                    x1s = x1all[:, b, :]
                    src = x2[b, 8 * i : 8 * i + 8].rearrange(
                        "(so si) c p -> (si c) so p", si=2
                    )
                    # The very last tile's chain is the kernel tail: q-slice
                    # its compute so the final dependency chain is one
                    # 441-wide chunk instead of a whole 1764-wide slice.
                    last_tile = b == BL - 1 and i == NBIG - 1
                    if last_tile:
                        # finest DMA granularity so the tail chain starts
                        # on the first pair
                        for so in range(NSO):
                            nc.sync.dma_start(x2t[:, so, :], src[:, so, :])
                    else:
                        # 902KB two-pair DMAs: fewer completion sems and
                        # longer descriptor runs for the sustained stream
                        for h in range(NSO // 2):
                            nc.sync.dma_start(
                                x2t[:, 2 * h : 2 * h + 2, :],
                                src[:, 2 * h : 2 * h + 2, :],
                            )
                    ot = None
                    if last_tile:
                        ot = outp.tile([S, HW], bf16, name="ot", tag="ot")
                    sq = sqp.tile([128, NSO, HW], bf16, tag="sq")
                    if not last_tile:
                        for so in range(NSO):
                            # in-place: x2t slice becomes diff
                            nc.vector.tensor_tensor(
                                x2t[:, so, :], x2t[:, so, :], x1s, sub
                            )
                            # two-pair squares batch once both subs are done
                            if so % 2 == 1:
                                hs = slice(so - 1, so + 1)
                                if SQ_ENG[2 * i + so // 2] == "A":
                                    nc.scalar.activation(
                                        sq[:, hs, :], x2t[:, hs, :], Square
                                    )
                                else:
                                    nc.vector.tensor_tensor(
                                        sq[:, hs, :],
                                        x2t[:, hs, :],
                                        x2t[:, hs, :],
                                        mybir.AluOpType.mult,
                                    )
                                for soj in (so - 1, so):
                                    j = NSO * i + soj
                                    for q in range(NQ):
                                        nc.tensor.matmul(
                                            pst[q][:, :],
                                            mt[:, j, :],
                                            sq[:, soj, QOFF[q] : QOFF[q] + QWID[q]],
                                            start=False,
                                            stop=(j == NPAIR - 2),
                                        )
                    else:
                        for so in range(NSO):
                            j = NSO * i + so
                            for q in range(NQ):
                                qs = slice(QOFF[q], QOFF[q] + QWID[q])
                                nc.vector.tensor_tensor(
                                    x2t[:, so, qs], x2t[:, so, qs], x1s[:, qs], sub
                                )
                                if q % 2 == 0:
                                    nc.scalar.activation(
                                        sq[:, so, qs], x2t[:, so, qs], Square
                                    )
                                else:
                                    nc.vector.tensor_tensor(
                                        sq[:, so, qs],
                                        x2t[:, so, qs],
                                        x2t[:, so, qs],
                                        mybir.AluOpType.mult,
                                    )
                                nc.tensor.matmul(
                                    pst[q][:, :],
                                    mt[:, j, :],
                                    sq[:, so, qs],
                                    start=False,
                                    stop=(j == NPAIR - 2),
                                )
                                if so == NSO - 1:
                                    # quarter q is complete: sqrt + store now
                                    nc.scalar.activation(
                                        ot[:, qs], pst[q][:], Sqrt
                                    )
                                    nc.gpsimd.dma_start(
                                        out[b].rearrange("(s p) -> s p", s=S)[:, qs],
                                        ot[:, qs],
                                    )

                if b < BL - 1:
                    ot = outp.tile([S, HW], bf16, name="ot", tag="ot")
                    for q in range(NQ):
                        nc.scalar.activation(
                            ot[:, QOFF[q] : QOFF[q] + QWID[q]], pst[q][:], Sqrt
                        )
                    # store via the GpSimd SWDGE ring: it carries no loads
                    # (no stall coupling) and keeps the ~0.6us dispatch off
                    # the busy ACT queue
                    nc.gpsimd.dma_start(out[b].rearrange("(s p) -> s p", s=S), ot[:])

    try:
        _dedupe_ldweights(nc)
        nc.finalize()
    finally:
        bacc.get_activation_tables = _orig_tables
    return nc


def get_nc():
    if "nc" not in _cache:
        _cache["nc"] = _build_nc()
    return _cache["nc"]


def make_mask() -> np.ndarray:
    # mask[j, k, m] = 1 iff partition k of pair-tile j feeds output support m.
    # Pair j < 12 covers supports (2j, 2j+1): k < 64 -> 2j, k >= 64 -> 2j+1.
    # Pair 12 is the leftover single support 24 on partitions 0..63.
    import ml_dtypes

    mask = np.zeros((NPAIR, 128, S), dtype=ml_dtypes.bfloat16)
    for j in range(NPAIR - 1):
        mask[j, 0:64, 2 * j] = 1.0
        mask[j, 64:128, 2 * j + 1] = 1.0
    mask[NPAIR - 1, 0:64, S - 1] = 1.0
    return mask


def make_in_maps(x1: np.ndarray, x2: np.ndarray) -> list[dict]:
    import ml_dtypes

    bf16 = ml_dtypes.bfloat16
    x1 = np.ascontiguousarray(
        np.asarray(x1, dtype=np.float32).astype(bf16)
    ).reshape(B, C, HW)
    x2 = np.ascontiguousarray(
        np.asarray(x2, dtype=np.float32).astype(bf16)
    ).reshape(B, S, C, HW)
    mask = make_mask()
    maps = []
    for i in range(NCORES):
        sl = slice(i * BL, (i + 1) * BL)
        maps.append({"x1": x1[sl], "x2": x2[sl], "mask": mask})
    return maps


def gather_out(results: list[dict]) -> np.ndarray:
    return np.concatenate([np.asarray(r["out"]) for r in results], axis=0).astype(
        np.float32, copy=False
    )


def kernel(x1, x2) -> np.ndarray:
    from concourse.bass_utils import run_bass_kernel_spmd

    nc = get_nc()
    in_maps = make_in_maps(x1, x2)
    res = run_bass_kernel_spmd(nc, in_maps, list(range(NCORES)))
    return gather_out(res.results)
